# revision 29
# baseline (speedup 1.0000x reference)
"""Dense2DSpatialTransformer (bilinear warp, N(0,1) flow) on 8 TRN2 cores.

Data-parallel over batch: each core warps 2 of the 16 images.

Dense path (per 128-row block, full 1024 width):
  The bilinear gather over the +-4 integer-shift window is computed as a
  telescoped weighted sum with cumulative-clamp weights:
    inner (per candidate row r in [-4..4]):
        H_r = T_r[.+8] + sum_{u=-4..3} G_u o DX_r[.+u+4]
        G_u = clamp(u+1-dW, 0, 1),  DX[y,x] = PAD[y,x] - PAD[y,x+1]
    outer (telescoped over rows, descending):
        out = H_4 + sum_{r=-4..3} GV_r o (H_r - H_{r+1})
        GV_r = clamp(r+1-dH, 0, 1)
  Weights are 4x-rate tensor_scalar chains on DVE; the 72+8 per-pixel
  products are 2x-rate fp16 tensor_tensors split across DVE and Pool; ALL
  adds ride the otherwise-idle PE array as identity-matmul accumulations
  into PSUM; ACT drains PSUM to fp16 and does the fp32->fp16 converts.
  Edge-replicated fp16 pad (+4 each side) makes clipping a no-op.

Sparse fixup: pixels with |dH| or |dW| >= 4-margin (~300/core) are
  recomputed exactly on device via per-element indirect-DMA gathers and
  scattered over the dense result (host supplies index metadata only).
"""
import sys

for _p in ("/opt/trn_rl_repo", "/opt/trn_rl_repo/concourse",
           "/root/.axon_site/_ro/trn_rl_repo"):
    if _p not in sys.path:
        sys.path.insert(0, _p)

import numpy as np

import concourse.bass as bass
import concourse.bacc as bacc
import concourse.mybir as mybir
import concourse.tile as tile
from concourse.bass import IndirectOffsetOnAxis
from concourse.bass_utils import run_bass_kernel_spmd
from concourse.masks import make_identity

f32 = np.float32
FP = mybir.dt.float32
F16 = mybir.dt.float16
AL = mybir.AluOpType
AF = mybir.ActivationFunctionType

B, H, W = 16, 1024, 1024
NCORES = 8
BPC = B // NCORES           # images per core
PAD = 4
PP = H + 2 * PAD            # padded side (1032)
NRB = H // 128              # row blocks per image
HW = H * W
HALF = W // 2
MARGIN = f32(2.0 ** -11)    # host/device classification guard band
OUT_TAIL = 128


def _phase_pad(nc, tc, v):
    """Build fp16 edge-padded images + horizontal-diff field in DRAM."""
    img = nc._k["img"]
    pph = nc._k["pph"]
    dxp = nc._k["dxp"]
    with tc.tile_pool(name="pad", bufs=2) as pad:
        for b in range(BPC):
            for rb in range(NRB):
                r0 = rb * 128
                t32 = pad.tile([128, W], FP, tag="t32")
                nc.sync.dma_start(out=t32[:], in_=img[b, r0:r0 + 128, :])
                te = pad.tile([128, PP], F16, tag="te")
                nc.scalar.activation(out=te[:, PAD:PAD + W], in_=t32[:],
                                     func=AF.Copy)
                for k in range(PAD):
                    v.tensor_copy(out=te[:, k:k + 1], in_=te[:, PAD:PAD + 1])
                    v.tensor_copy(out=te[:, PAD + W + k:PAD + W + k + 1],
                                  in_=te[:, PAD + W - 1:PAD + W])
                nc.sync.dma_start(out=pph[b, PAD + r0:PAD + r0 + 128, :],
                                  in_=te[:])
                de = pad.tile([128, PP], F16, tag="de")
                v.tensor_tensor(out=de[:, 0:PP - 1], in0=te[:, 0:PP - 1],
                                in1=te[:, 1:PP], op=AL.subtract)
                v.tensor_copy(out=de[:, PP - 1:PP], in_=de[:, PP - 2:PP - 1])
                nc.sync.dma_start(out=dxp[b, PAD + r0:PAD + r0 + 128, :],
                                  in_=de[:])
            for k in range(PAD):
                nc.sync.dma_start(out=pph[b, k:k + 1, :],
                                  in_=pph[b, PAD:PAD + 1, :])
                nc.sync.dma_start(out=pph[b, PP - 1 - k:PP - k, :],
                                  in_=pph[b, PP - PAD - 1:PP - PAD, :])
                nc.sync.dma_start(out=dxp[b, k:k + 1, :],
                                  in_=dxp[b, PAD:PAD + 1, :])
                nc.sync.dma_start(out=dxp[b, PP - 1 - k:PP - k, :],
                                  in_=dxp[b, PP - PAD - 1:PP - PAD, :])


def _ovl(ap, dims):
    """Custom free-dim [stride, count] view of an AP (overlapping allowed)."""
    import bass_rust
    a = ap.copy()
    a.ap = bass_rust.VecI64Pair([list(a.ap[0])] + [list(d) for d in dims])
    return a


# rows whose unified product stack runs on Pool instead of DVE
POOL_ROWS = ()


def _phase_dense(nc, tc, v, g, ident, biases):
    """16 blocks of the telescoped bilinear warp.

    Inner telescope uses complement weights so ALL clamp tails fuse:
      H_r = T_r[.-4] - sum_{u=-4..3} r2_u o DX_r[.+u+4]
      r2_u = 1 - clamp(u+1-dW, 0, 1) = relu(1 - relu(u+1-dW))
    built as ACT Relu (r1) + one fused DVE ts (min,sub -> -r2); negated
    weights make the PE accumulation come out with the right sign.
    """
    pph = nc._k["pph"]
    dxp = nc._k["dxp"]
    flow4 = nc._k["flow4"]
    out3 = nc._k["out3"]
    with tc.tile_pool(name="tp", bufs=1) as tpool, \
         tc.tile_pool(name="lds", bufs=2) as lds, \
         tc.tile_pool(name="wts", bufs=1) as wts, \
         tc.tile_pool(name="vp", bufs=2) as vp, \
         tc.tile_pool(name="hp", bufs=2) as hpool, \
         tc.tile_pool(name="fl", bufs=1) as flp, \
         tc.tile_pool(name="prod", bufs=3) as pp_, \
         tc.tile_pool(name="dp", bufs=1) as dp_, \
         tc.tile_pool(name="ob", bufs=1) as ob, \
         tc.psum_pool(name="ps", bufs=2) as psp, \
         tc.psum_pool(name="pso", bufs=2) as psop:

        def emit_outer_pool(st):
            """H-diff stacks on the idle Pool engine (loose timing)."""
            hstack = st["hstack"]
            for half in range(2):
                s = half * 4
                dstk = dp_.tile([128, 4, W], F16, tag=f"dstk{half}",
                                name=f"dstk{half}")
                g.tensor_tensor(out=dstk[:], in0=hstack[:, s + 1:s + 5, :],
                                in1=hstack[:, s:s + 4, :], op=AL.subtract)
                st[f"d{half}"] = dstk

        def emit_outer_mul(st):
            nr2v = st["nr2v"]
            for half in range(2):
                postk = pp_.tile([128, 4, W], F16, tag="pstk")
                v.tensor_tensor(out=postk[:], in0=nr2v[:, half * 4:half * 4 + 4, :],
                                in1=st[f"d{half}"][:], op=AL.mult)
                st[f"p{half}"] = postk

        def emit_outer_fin(st):
            """psOUT accumulation + drain + store."""
            hstack, b, r0 = st["hstack"], st["b"], st["r0"]
            psOUT = psop.tile([128, W], FP, tag="psOUT")
            for h in range(2):
                nc.tensor.matmul(
                    psOUT[:, h * HALF:(h + 1) * HALF], ident[:],
                    hstack[:, 8, h * HALF:(h + 1) * HALF],
                    start=True, stop=False)
            for half in range(2):
                postk = st[f"p{half}"]
                for j in range(4):
                    for h in range(2):
                        nc.tensor.matmul(
                            psOUT[:, h * HALF:(h + 1) * HALF], ident[:],
                            postk[:, j, h * HALF:(h + 1) * HALF],
                            start=False,
                            stop=(half == 1 and j == 3 and h == 1))
            out16 = ob.tile([128, W], F16, tag="out16")
            nc.scalar.activation(out=out16[:], in_=psOUT[:], func=AF.Copy)
            nc.sync.dma_start(out=out3[b, r0:r0 + 128, :], in_=out16[:])

        prev = None
        for b in range(BPC):
            for rb in range(NRB):
                r0 = rb * 128
                T = {}
                DXe = {}
                for r in range(-4, 5):
                    rs = r0 + r + PAD
                    t = tpool.tile([128, W], F16, tag=f"T{r}")
                    nc.sync.dma_start(out=t[:],
                                      in_=pph[b, rs:rs + 128, 0:W])
                    T[r] = t
                    d = lds.tile([128, PP], F16, tag=f"DXe{r}")
                    nc.sync.dma_start(out=d[:], in_=dxp[b, rs:rs + 128, :])
                    DXe[r] = d
                dh32 = flp.tile([128, W], FP, tag="dh32")
                nc.sync.dma_start(out=dh32[:], in_=flow4[b, 0, r0:r0 + 128, :])
                dw32 = flp.tile([128, W], FP, tag="dw32")
                nc.sync.dma_start(out=dw32[:], in_=flow4[b, 1, r0:r0 + 128, :])

                # negated fp16 flow (ACT converts + negates in one op)
                nRW = wts.tile([128, W], F16, tag="nRW")
                nc.scalar.activation(out=nRW[:], in_=dw32[:], func=AF.Copy,
                                     scale=-1.0)
                nRH = wts.tile([128, W], F16, tag="nRH")
                nc.scalar.activation(out=nRH[:], in_=dh32[:], func=AF.Copy,
                                     scale=-1.0)

                # negated complement weights -r2_u = min(relu(u+1-dW),1) - 1,
                # all 8 u in natural order in one stack. ACT writes the relu
                # halves into a scratch; one stacked non-inplace ts finishes.
                nr2a = wts.tile([128, 8, W], F16, tag="nr2a")
                nr2v = vp.tile([128, 8, W], F16, tag="nr2v")
                raw = wts.tile([128, 4, W], F16, tag="raw")
                for grp, (dst, src, cof) in enumerate((
                        (nr2a, nRW, lambda j: j - 3),      # u+1, u=j-4
                        (nr2v, nRH, lambda j: 4 - j))):    # r+1, r=3-j
                    for half in range(2):
                        scr = raw
                        for k in range(4):
                            j = half * 4 + k
                            nc.scalar.activation(
                                out=scr[:, k, :], in_=src[:], func=AF.Relu,
                                bias=biases[cof(j)][:, 0:1], scale=1.0)
                        v.tensor_scalar(out=dst[:, half * 4:half * 4 + 4, :],
                                        in0=scr[:], scalar1=1.0, scalar2=1.0,
                                        op0=AL.min, op1=AL.subtract)

                hstack = hpool.tile([128, 9, W], F16, tag="hstack")
                for i, r in enumerate(range(4, -5, -1)):   # descending rows
                    psA = psp.tile([128, W], FP, tag="psA")
                    # base term T_r[.-4] (tile col 0)
                    for h in range(2):
                        nc.tensor.matmul(
                            psA[:, h * HALF:(h + 1) * HALF], ident[:],
                            T[r][:, h * HALF:(h + 1) * HALF],
                            start=True, stop=False)
                    # half-size product stacks (finer PE pipelining)
                    for half in range(2):
                        pstk = pp_.tile([128, 4, W], F16, tag="pstk")
                        v.tensor_tensor(
                            out=pstk[:], in0=nr2a[:, half * 4:half * 4 + 4, :],
                            in1=_ovl(DXe[r][:, half * 4:], [[1, 4], [1, W]]),
                            op=AL.mult)
                        for j in range(4):
                            for h in range(2):
                                nc.tensor.matmul(
                                    psA[:, h * HALF:(h + 1) * HALF], ident[:],
                                    pstk[:, j, h * HALF:(h + 1) * HALF],
                                    start=False,
                                    stop=(half == 1 and j == 3 and h == 1))
                    nc.scalar.activation(out=hstack[:, i, :], in_=psA[:],
                                         func=AF.Copy)
                    # overlap the previous block's tail with this block's body
                    if prev is not None:
                        if i == 1:
                            emit_outer_pool(prev)
                        elif i == 6:
                            emit_outer_mul(prev)
                        elif i == 7:
                            emit_outer_fin(prev)
                            prev = None
                prev = {"hstack": hstack, "nr2v": nr2v, "b": b, "r0": r0}
        emit_outer_pool(prev)
        emit_outer_mul(prev)
        emit_outer_fin(prev)


def _phase_sparse(nc, tc, v, g, nout):
    """Exact recompute of outlier pixels via indirect-DMA gathers."""
    flowf = nc._k["flowf"]
    pphf = nc._k["pphf"]
    outf = nc._k["outf"]
    k = nc._k
    NCH = nout // 128
    with tc.tile_pool(name="fix", bufs=1) as fx:
        def load_aux(d, dt, name):
            t = fx.tile([128, NCH], dt, tag=name)
            nc.sync.dma_start(
                out=t[:], in_=d.ap().rearrange("(p f) -> p f", p=128))
            return t

        opos_s = load_aux(k["opos_d"], mybir.dt.int32, "opos")
        odh_s = load_aux(k["odh_d"], mybir.dt.int32, "odh")
        odw_s = load_aux(k["odw_d"], mybir.dt.int32, "odw")
        oh_s = load_aux(k["oh_d"], FP, "oh")
        oh1_s = load_aux(k["oh1_d"], FP, "oh1")
        ow_s = load_aux(k["ow_d"], FP, "ow")
        ow1_s = load_aux(k["ow1_d"], FP, "ow1")
        obase_s = load_aux(k["obase_d"], FP, "obase")

        dhv = fx.tile([128, NCH], FP, tag="dhv")
        dwv = fx.tile([128, NCH], FP, tag="dwv")
        for c in range(NCH):
            g.indirect_dma_start(
                out=dhv[:, c:c + 1], out_offset=None, in_=flowf[:, None],
                in_offset=IndirectOffsetOnAxis(ap=odh_s[:, c:c + 1], axis=0))
            g.indirect_dma_start(
                out=dwv[:, c:c + 1], out_offset=None, in_=flowf[:, None],
                in_offset=IndirectOffsetOnAxis(ap=odw_s[:, c:c + 1], axis=0))

        def fields(dv, hb, hb1, pfx):
            """y = (dv + hb) + 1 ; R = y - hb1 ; d = floor(R)+1-R."""
            yt = fx.tile([128, NCH], FP, tag=f"{pfx}y")
            v.tensor_tensor(out=yt[:], in0=dv[:], in1=hb[:], op=AL.add)
            v.tensor_scalar(out=yt[:], in0=yt[:], scalar1=1.0,
                            scalar2=None, op0=AL.add)
            Rt = fx.tile([128, NCH], FP, tag=f"{pfx}R")
            v.tensor_tensor(out=Rt[:], in0=yt[:], in1=hb1[:], op=AL.subtract)
            St = fx.tile([128, NCH], FP, tag=f"{pfx}S")
            gt = fx.tile([128, NCH], FP, tag=f"{pfx}g")
            v.tensor_scalar(out=St[:], in0=Rt[:], scalar1=-6.0,
                            scalar2=None, op0=AL.is_ge)
            for s in range(-5, 7):
                v.tensor_scalar(out=gt[:], in0=Rt[:], scalar1=float(s),
                                scalar2=None, op0=AL.is_ge)
                v.tensor_tensor(out=St[:], in0=St[:], in1=gt[:], op=AL.add)
            dt_ = fx.tile([128, NCH], FP, tag=f"{pfx}d")
            v.tensor_scalar(out=dt_[:], in0=St[:], scalar1=-6.0,
                            scalar2=None, op0=AL.add)
            v.tensor_tensor(out=dt_[:], in0=dt_[:], in1=Rt[:], op=AL.subtract)
            return yt, dt_

        yv, dhw = fields(dhv, oh_s, oh1_s, "fh")
        ywv, dww = fields(dwv, ow_s, ow1_s, "fw")

        # base corner in padded coords: row = clamp(y+dh+2, 0, PP-2), col same
        rowp = fx.tile([128, NCH], FP, tag="rowp")
        v.tensor_tensor(out=rowp[:], in0=yv[:], in1=dhw[:], op=AL.add)
        v.tensor_scalar(out=rowp[:], in0=rowp[:], scalar1=2.0, scalar2=0.0,
                        op0=AL.add, op1=AL.max)
        v.tensor_scalar(out=rowp[:], in0=rowp[:], scalar1=float(PP - 2),
                        scalar2=float(PP), op0=AL.min, op1=AL.mult)
        colp = fx.tile([128, NCH], FP, tag="colp")
        v.tensor_tensor(out=colp[:], in0=ywv[:], in1=dww[:], op=AL.add)
        v.tensor_scalar(out=colp[:], in0=colp[:], scalar1=2.0, scalar2=0.0,
                        op0=AL.add, op1=AL.max)
        v.tensor_scalar(out=colp[:], in0=colp[:], scalar1=float(PP - 2),
                        scalar2=None, op0=AL.min)
        af = fx.tile([128, NCH], FP, tag="af")
        v.tensor_tensor(out=af[:], in0=rowp[:], in1=colp[:], op=AL.add)
        v.tensor_tensor(out=af[:], in0=af[:], in1=obase_s[:], op=AL.add)

        vals = {}
        afo = fx.tile([128, NCH], FP, tag="afo")
        for (cn, doff) in (("v00", 0.0), ("v10", 1.0),
                           ("v01", float(PP)), ("v11", float(PP + 1))):
            ai = fx.tile([128, NCH], mybir.dt.int32, tag=f"ai{cn}")
            if doff == 0.0:
                v.tensor_copy(out=ai[:], in_=af[:])
            else:
                v.tensor_scalar(out=afo[:], in0=af[:], scalar1=doff,
                                scalar2=None, op0=AL.add)
                v.tensor_copy(out=ai[:], in_=afo[:])
            vt16 = fx.tile([128, NCH], F16, tag=f"{cn}h")
            for c in range(NCH):
                g.indirect_dma_start(
                    out=vt16[:, c:c + 1], out_offset=None, in_=pphf[:, None],
                    in_offset=IndirectOffsetOnAxis(ap=ai[:, c:c + 1], axis=0))
            vt = fx.tile([128, NCH], FP, tag=cn)
            v.tensor_copy(out=vt[:], in_=vt16[:])
            vals[cn] = vt

        omw_f = fx.tile([128, NCH], FP, tag="omwf")
        v.tensor_scalar(out=omw_f[:], in0=dww[:], scalar1=-1.0,
                        scalar2=1.0, op0=AL.mult, op1=AL.add)
        omh_f = fx.tile([128, NCH], FP, tag="omhf")
        v.tensor_scalar(out=omh_f[:], in0=dhw[:], scalar1=-1.0,
                        scalar2=1.0, op0=AL.mult, op1=AL.add)
        wt = fx.tile([128, NCH], FP, tag="wtf")
        accf = fx.tile([128, NCH], FP, tag="accf")
        t3 = fx.tile([128, NCH], FP, tag="t3")
        v.tensor_tensor(out=wt[:], in0=dhw[:], in1=dww[:], op=AL.mult)
        v.tensor_tensor(out=accf[:], in0=vals["v00"][:], in1=wt[:], op=AL.mult)
        v.tensor_tensor(out=wt[:], in0=dhw[:], in1=omw_f[:], op=AL.mult)
        v.tensor_tensor(out=t3[:], in0=vals["v10"][:], in1=wt[:], op=AL.mult)
        v.tensor_tensor(out=accf[:], in0=accf[:], in1=t3[:], op=AL.add)
        v.tensor_tensor(out=wt[:], in0=omh_f[:], in1=dww[:], op=AL.mult)
        v.tensor_tensor(out=t3[:], in0=vals["v01"][:], in1=wt[:], op=AL.mult)
        v.tensor_tensor(out=accf[:], in0=accf[:], in1=t3[:], op=AL.add)
        v.tensor_tensor(out=wt[:], in0=omw_f[:], in1=omh_f[:], op=AL.mult)
        v.tensor_tensor(out=t3[:], in0=vals["v11"][:], in1=wt[:], op=AL.mult)
        v.tensor_tensor(out=accf[:], in0=accf[:], in1=t3[:], op=AL.add)

        acc16 = fx.tile([128, NCH], F16, tag="acc16")
        v.tensor_copy(out=acc16[:], in_=accf[:])
        for c in range(NCH):
            g.indirect_dma_start(
                out=outf[:, None],
                out_offset=IndirectOffsetOnAxis(ap=opos_s[:, c:c + 1], axis=0),
                in_=acc16[:, c:c + 1], in_offset=None)


def _build_program(nout):
    nc = bacc.Bacc("TRN2", target_bir_lowering=False, debug=False,
                   enable_asserts=False, num_devices=NCORES)

    img_d = nc.dram_tensor("img", [BPC, H, W], FP, kind="ExternalInput")
    flow_d = nc.dram_tensor("flow", [BPC * 2 * HW], FP, kind="ExternalInput")
    opos_d = nc.dram_tensor("opos", [nout], mybir.dt.int32, kind="ExternalInput")
    odh_d = nc.dram_tensor("odh", [nout], mybir.dt.int32, kind="ExternalInput")
    odw_d = nc.dram_tensor("odw", [nout], mybir.dt.int32, kind="ExternalInput")
    oh_d = nc.dram_tensor("oh", [nout], FP, kind="ExternalInput")
    oh1_d = nc.dram_tensor("oh1", [nout], FP, kind="ExternalInput")
    ow_d = nc.dram_tensor("ow", [nout], FP, kind="ExternalInput")
    ow1_d = nc.dram_tensor("ow1", [nout], FP, kind="ExternalInput")
    obase_d = nc.dram_tensor("obase", [nout], FP, kind="ExternalInput")
    padh_d = nc.dram_tensor("padh", [BPC * PP * PP], F16, kind="Internal")
    dxp_d = nc.dram_tensor("dxp", [BPC * PP * PP], F16, kind="Internal")
    out_d = nc.dram_tensor("out", [BPC * HW + OUT_TAIL], F16,
                           kind="ExternalOutput")

    flowf = flow_d.ap()
    pphf = padh_d.ap()
    dxf = dxp_d.ap()
    outf = out_d.ap()
    nc._k = {
        "img": img_d.ap(),
        "flowf": flowf,
        "flow4": flowf.rearrange("(b c h w) -> b c h w", b=BPC, c=2, h=H, w=W),
        "pphf": pphf,
        "pph": pphf.rearrange("(b h w) -> b h w", b=BPC, h=PP, w=PP),
        "dxp": dxf.rearrange("(b h w) -> b h w", b=BPC, h=PP, w=PP),
        "outf": outf,
        "out3": outf[0:BPC * HW].rearrange("(b h w) -> b h w", b=BPC, h=H, w=W),
        "opos_d": opos_d, "odh_d": odh_d, "odw_d": odw_d, "oh_d": oh_d,
        "oh1_d": oh1_d, "ow_d": ow_d, "ow1_d": ow1_d, "obase_d": obase_d,
    }

    v = nc.vector
    g = nc.gpsimd

    with tile.TileContext(nc) as tc:
        with tc.tile_pool(name="pers", bufs=1) as pers:
            ident = pers.tile([128, 128], F16, tag="ident")
            make_identity(nc, ident[:])
            biases = {}
            for c in range(-3, 5):
                bt = pers.tile([128, 1], FP, tag=f"bias{c}", name=f"bias{c}")
                v.memset(bt[:], float(c))
                biases[c] = bt
            _phase_pad(nc, tc, v)
            _phase_dense(nc, tc, v, g, ident, biases)
            _phase_sparse(nc, tc, v, g, nout)

    nc.compile()
    return nc


_PROGRAM_CACHE = {}


def _get_program(nout):
    if nout not in _PROGRAM_CACHE:
        _PROGRAM_CACHE[nout] = _build_program(nout)
    return _PROGRAM_CACHE[nout]


def _host_metadata(dH, dW):
    """Outlier positions for one image, mirroring the reference fp32 math."""
    h = (np.arange(H, dtype=f32)[:, None] * np.ones((1, W), f32))
    w = (np.ones((H, 1), f32) * np.arange(W, dtype=f32)[None, :])
    y = ((dH + h).astype(f32) + f32(1.0)).astype(f32)
    yw = ((dW + w).astype(f32) + f32(1.0)).astype(f32)
    R = (y - (h + f32(1.0))).astype(f32)
    Rw = (yw - (w + f32(1.0))).astype(f32)
    inl = ((R >= f32(-4.0) + MARGIN) & (R < f32(4.0) - MARGIN)
           & (Rw >= f32(-4.0) + MARGIN) & (Rw < f32(4.0) - MARGIN))
    oy, ox = np.where(~inl)
    return oy.astype(np.int64), ox.astype(np.int64)


def _prepare(input1, input2):
    input1 = np.asarray(input1)
    input2 = np.asarray(input2)
    assert input1.shape == (B, 1, H, W) and input2.shape == (B, 2, H, W)

    metas = []
    max_n = 1
    for c in range(NCORES):
        rows = []
        for bl in range(BPC):
            bglob = c * BPC + bl
            oy, ox = _host_metadata(input2[bglob, 0], input2[bglob, 1])
            rows.append((bl, oy, ox))
        n = sum(len(oy) for _, oy, _ in rows)
        max_n = max(max_n, n)
        metas.append(rows)
    nout = max(128, ((max_n + 127) // 128) * 128)

    nc = _get_program(nout)

    in_maps = []
    for c in range(NCORES):
        imgs = input1[c * BPC:(c + 1) * BPC, 0]
        flow = input2[c * BPC:(c + 1) * BPC]
        opos = np.full(nout, BPC * HW, np.int32)
        odh = np.zeros(nout, np.int32)
        odw = np.full(nout, HW, np.int32)
        oh = np.zeros(nout, f32)
        ow = np.zeros(nout, f32)
        obase = np.zeros(nout, f32)
        k = 0
        for bl, oy, ox in metas[c]:
            n = len(oy)
            opos[k:k + n] = (bl * HW + oy * W + ox).astype(np.int32)
            odh[k:k + n] = (bl * 2 * HW + oy * W + ox).astype(np.int32)
            odw[k:k + n] = (bl * 2 * HW + HW + oy * W + ox).astype(np.int32)
            oh[k:k + n] = oy.astype(f32)
            ow[k:k + n] = ox.astype(f32)
            obase[k:k + n] = f32(bl * PP * PP)
            k += n
        in_maps.append({
            "img": np.ascontiguousarray(imgs),
            "flow": np.ascontiguousarray(flow.reshape(-1)),
            "opos": opos, "odh": odh, "odw": odw,
            "oh": oh, "oh1": (oh + f32(1.0)).astype(f32),
            "ow": ow, "ow1": (ow + f32(1.0)).astype(f32),
            "obase": obase,
        })

    return nc, in_maps


def _assemble(results):
    out = np.empty((B, 1, H, W), f32)
    for c in range(NCORES):
        o = results[c]["out"][:BPC * HW].astype(f32).reshape(BPC, H, W)
        out[c * BPC:(c + 1) * BPC, 0] = o
    return out


def kernel(input1, input2):
    nc, in_maps = _prepare(input1, input2)
    res = run_bass_kernel_spmd(nc, in_maps, core_ids=list(range(NCORES)))
    return _assemble(res.results)


# revision 30
# speedup vs baseline: 1.3280x; 1.3280x over previous
"""Dense2DSpatialTransformer (bilinear warp, N(0,1) flow) on 8 TRN2 cores.

Data-parallel over batch: each core warps 2 of the 16 images.

Dense path (per 128-row block, full 1024 width):
  The bilinear gather over the +-4 integer-shift window is computed as a
  telescoped weighted sum with cumulative-clamp weights:
    inner (per candidate row r in [-4..4]):
        H_r = T_r[.+8] + sum_{u=-4..3} G_u o DX_r[.+u+4]
        G_u = clamp(u+1-dW, 0, 1),  DX[y,x] = PAD[y,x] - PAD[y,x+1]
    outer (telescoped over rows, descending):
        out = H_4 + sum_{r=-4..3} GV_r o (H_r - H_{r+1})
        GV_r = clamp(r+1-dH, 0, 1)
  Weights are 4x-rate tensor_scalar chains on DVE; the 72+8 per-pixel
  products are 2x-rate fp16 tensor_tensors split across DVE and Pool; ALL
  adds ride the otherwise-idle PE array as identity-matmul accumulations
  into PSUM; ACT drains PSUM to fp16 and does the fp32->fp16 converts.
  Edge-replicated fp16 pad (+4 each side) makes clipping a no-op.

Sparse fixup: pixels with |dH| or |dW| >= 4-margin (~300/core) are
  recomputed exactly on device via per-element indirect-DMA gathers and
  scattered over the dense result (host supplies index metadata only).
"""
import sys

for _p in ("/opt/trn_rl_repo", "/opt/trn_rl_repo/concourse",
           "/root/.axon_site/_ro/trn_rl_repo"):
    if _p not in sys.path:
        sys.path.insert(0, _p)

import numpy as np

import concourse.bass as bass
import concourse.bacc as bacc
import concourse.mybir as mybir
import concourse.tile as tile
from concourse.bass import IndirectOffsetOnAxis
from concourse.bass_utils import run_bass_kernel_spmd
from concourse.masks import make_identity

f32 = np.float32
FP = mybir.dt.float32
F16 = mybir.dt.float16
AL = mybir.AluOpType
AF = mybir.ActivationFunctionType

B, H, W = 16, 1024, 1024
NCORES = 8
BPC = B // NCORES           # images per core
PAD = 4
PP = H + 2 * PAD            # padded side (1032)
NRB = H // 128              # row blocks per image
HW = H * W
HALF = W // 2
MARGIN = f32(2.0 ** -11)    # host/device classification guard band
OUT_TAIL = 128


def _phase_pad(nc, tc, v):
    """Build fp16 edge-padded images + horizontal-diff field in DRAM."""
    img = nc._k["img"]
    pph = nc._k["pph"]
    dxp = nc._k["dxp"]
    with tc.tile_pool(name="pad", bufs=2) as pad:
        for b in range(BPC):
            for rb in range(NRB):
                r0 = rb * 128
                t32 = pad.tile([128, W], FP, tag="t32")
                nc.sync.dma_start(out=t32[:], in_=img[b, r0:r0 + 128, :])
                te = pad.tile([128, PP], F16, tag="te")
                nc.scalar.activation(out=te[:, PAD:PAD + W], in_=t32[:],
                                     func=AF.Copy)
                for k in range(PAD):
                    v.tensor_copy(out=te[:, k:k + 1], in_=te[:, PAD:PAD + 1])
                    v.tensor_copy(out=te[:, PAD + W + k:PAD + W + k + 1],
                                  in_=te[:, PAD + W - 1:PAD + W])
                nc.sync.dma_start(out=pph[b, PAD + r0:PAD + r0 + 128, :],
                                  in_=te[:])
                de = pad.tile([128, PP], F16, tag="de")
                v.tensor_tensor(out=de[:, 0:PP - 1], in0=te[:, 0:PP - 1],
                                in1=te[:, 1:PP], op=AL.subtract)
                v.tensor_copy(out=de[:, PP - 1:PP], in_=de[:, PP - 2:PP - 1])
                nc.sync.dma_start(out=dxp[b, PAD + r0:PAD + r0 + 128, :],
                                  in_=de[:])
            for k in range(PAD):
                nc.sync.dma_start(out=pph[b, k:k + 1, :],
                                  in_=pph[b, PAD:PAD + 1, :])
                nc.sync.dma_start(out=pph[b, PP - 1 - k:PP - k, :],
                                  in_=pph[b, PP - PAD - 1:PP - PAD, :])
                nc.sync.dma_start(out=dxp[b, k:k + 1, :],
                                  in_=dxp[b, PAD:PAD + 1, :])
                nc.sync.dma_start(out=dxp[b, PP - 1 - k:PP - k, :],
                                  in_=dxp[b, PP - PAD - 1:PP - PAD, :])


def _ovl(ap, dims):
    """Custom free-dim [stride, count] view of an AP (overlapping allowed)."""
    import bass_rust
    a = ap.copy()
    a.ap = bass_rust.VecI64Pair([list(a.ap[0])] + [list(d) for d in dims])
    return a


# rows whose unified product stack runs on Pool instead of DVE
POOL_ROWS = ()


def _phase_dense(nc, tc, v, g, ident, biases):
    """16 blocks of the telescoped bilinear warp.

    Inner telescope uses complement weights so ALL clamp tails fuse:
      H_r = T_r[.-4] - sum_{u=-4..3} r2_u o DX_r[.+u+4]
      r2_u = 1 - clamp(u+1-dW, 0, 1) = relu(1 - relu(u+1-dW))
    built as ACT Relu (r1) + one fused DVE ts (min,sub -> -r2); negated
    weights make the PE accumulation come out with the right sign.
    """
    pph = nc._k["pph"]
    dxp = nc._k["dxp"]
    flow4 = nc._k["flow4"]
    out3 = nc._k["out3"]
    with tc.tile_pool(name="tp", bufs=1) as tpool, \
         tc.tile_pool(name="lds", bufs=2) as lds, \
         tc.tile_pool(name="wts", bufs=1) as wts, \
         tc.tile_pool(name="vp", bufs=2) as vp, \
         tc.tile_pool(name="hp", bufs=2) as hpool, \
         tc.tile_pool(name="fl", bufs=2) as flp, \
         tc.tile_pool(name="prod", bufs=4) as pp_, \
         tc.tile_pool(name="ob", bufs=2) as ob, \
         tc.psum_pool(name="ps", bufs=2) as psp, \
         tc.psum_pool(name="pso", bufs=2) as psop:

        def emit_outer(st):
            """Tail of a block: out = H_{-4} - sum_j r2v_j o D_j."""
            hstack, nr2v, b, r0 = st
            psOUT = psop.tile([128, W], FP, tag="psOUT")
            for h in range(2):
                nc.tensor.matmul(
                    psOUT[:, h * HALF:(h + 1) * HALF], ident[:],
                    hstack[:, 8, h * HALF:(h + 1) * HALF],
                    start=True, stop=False)
            for half in range(2):
                s = half * 4
                dstk = pp_.tile([128, 4, W], F16, tag="pstk")
                v.tensor_tensor(out=dstk[:], in0=hstack[:, s + 1:s + 5, :],
                                in1=hstack[:, s:s + 4, :], op=AL.subtract)
                postk = pp_.tile([128, 4, W], F16, tag="pstk")
                v.tensor_tensor(out=postk[:], in0=nr2v[:, s:s + 4, :],
                                in1=dstk[:], op=AL.mult)
                for j in range(4):
                    for h in range(2):
                        nc.tensor.matmul(
                            psOUT[:, h * HALF:(h + 1) * HALF], ident[:],
                            postk[:, j, h * HALF:(h + 1) * HALF],
                            start=False,
                            stop=(half == 1 and j == 3 and h == 1))
            out16 = ob.tile([128, W], F16, tag="out16")
            nc.scalar.activation(out=out16[:], in_=psOUT[:], func=AF.Copy)
            nc.sync.dma_start(out=out3[b, r0:r0 + 128, :], in_=out16[:])

        prev = None
        for b in range(BPC):
            for rb in range(NRB):
                r0 = rb * 128
                T = {}
                DXe = {}
                for r in range(-4, 5):
                    rs = r0 + r + PAD
                    t = tpool.tile([128, W], F16, tag=f"T{r}")
                    nc.sync.dma_start(out=t[:],
                                      in_=pph[b, rs:rs + 128, 0:W])
                    T[r] = t
                    d = lds.tile([128, PP], F16, tag=f"DXe{r}")
                    nc.sync.dma_start(out=d[:], in_=dxp[b, rs:rs + 128, :])
                    DXe[r] = d
                dh32 = flp.tile([128, W], FP, tag="dh32")
                nc.sync.dma_start(out=dh32[:], in_=flow4[b, 0, r0:r0 + 128, :])
                dw32 = flp.tile([128, W], FP, tag="dw32")
                nc.sync.dma_start(out=dw32[:], in_=flow4[b, 1, r0:r0 + 128, :])

                # negated fp16 flow (ACT converts + negates in one op)
                nRW = wts.tile([128, W], F16, tag="nRW")
                nc.scalar.activation(out=nRW[:], in_=dw32[:], func=AF.Copy,
                                     scale=-1.0)
                nRH = wts.tile([128, W], F16, tag="nRH")
                nc.scalar.activation(out=nRH[:], in_=dh32[:], func=AF.Copy,
                                     scale=-1.0)

                # negated complement weights -r2_u = min(relu(u+1-dW),1) - 1,
                # all 8 u in natural order in one stack. ACT writes the relu
                # halves into a scratch; one stacked non-inplace ts finishes.
                nr2a = wts.tile([128, 8, W], F16, tag="nr2a")
                nr2v = vp.tile([128, 8, W], F16, tag="nr2v")
                raw = wts.tile([128, 4, W], F16, tag="raw")
                for grp, (dst, src, cof) in enumerate((
                        (nr2a, nRW, lambda j: j - 3),      # u+1, u=j-4
                        (nr2v, nRH, lambda j: 4 - j))):    # r+1, r=3-j
                    for half in range(2):
                        scr = raw
                        for k in range(4):
                            j = half * 4 + k
                            nc.scalar.activation(
                                out=scr[:, k, :], in_=src[:], func=AF.Relu,
                                bias=biases[cof(j)][:, 0:1], scale=1.0)
                        v.tensor_scalar(out=dst[:, half * 4:half * 4 + 4, :],
                                        in0=scr[:], scalar1=1.0, scalar2=1.0,
                                        op0=AL.min, op1=AL.subtract)

                hstack = hpool.tile([128, 9, W], F16, tag="hstack")
                for i, r in enumerate(range(4, -5, -1)):   # descending rows
                    psA = psp.tile([128, W], FP, tag="psA")
                    # base term T_r[.-4] (tile col 0)
                    for h in range(2):
                        nc.tensor.matmul(
                            psA[:, h * HALF:(h + 1) * HALF], ident[:],
                            T[r][:, h * HALF:(h + 1) * HALF],
                            start=True, stop=False)
                    # half-size product stacks (finer PE pipelining)
                    for half in range(2):
                        pstk = pp_.tile([128, 4, W], F16, tag="pstk")
                        v.tensor_tensor(
                            out=pstk[:], in0=nr2a[:, half * 4:half * 4 + 4, :],
                            in1=_ovl(DXe[r][:, half * 4:], [[1, 4], [1, W]]),
                            op=AL.mult)
                        for j in range(4):
                            for h in range(2):
                                nc.tensor.matmul(
                                    psA[:, h * HALF:(h + 1) * HALF], ident[:],
                                    pstk[:, j, h * HALF:(h + 1) * HALF],
                                    start=False,
                                    stop=(half == 1 and j == 3 and h == 1))
                    nc.scalar.activation(out=hstack[:, i, :], in_=psA[:],
                                         func=AF.Copy)
                    # overlap the previous block's tail with this block's body
                    if i == 1 and prev is not None:
                        emit_outer(prev)
                        prev = None
                prev = (hstack, nr2v, b, r0)
        emit_outer(prev)


def _phase_sparse(nc, tc, v, g, nout):
    """Exact recompute of outlier pixels via indirect-DMA gathers."""
    flowf = nc._k["flowf"]
    pphf = nc._k["pphf"]
    outf = nc._k["outf"]
    k = nc._k
    NCH = nout // 128
    with tc.tile_pool(name="fix", bufs=1) as fx:
        def load_aux(d, dt, name):
            t = fx.tile([128, NCH], dt, tag=name)
            nc.sync.dma_start(
                out=t[:], in_=d.ap().rearrange("(p f) -> p f", p=128))
            return t

        opos_s = load_aux(k["opos_d"], mybir.dt.int32, "opos")
        odh_s = load_aux(k["odh_d"], mybir.dt.int32, "odh")
        odw_s = load_aux(k["odw_d"], mybir.dt.int32, "odw")
        oh_s = load_aux(k["oh_d"], FP, "oh")
        oh1_s = load_aux(k["oh1_d"], FP, "oh1")
        ow_s = load_aux(k["ow_d"], FP, "ow")
        ow1_s = load_aux(k["ow1_d"], FP, "ow1")
        obase_s = load_aux(k["obase_d"], FP, "obase")

        dhv = fx.tile([128, NCH], FP, tag="dhv")
        dwv = fx.tile([128, NCH], FP, tag="dwv")
        for c in range(NCH):
            g.indirect_dma_start(
                out=dhv[:, c:c + 1], out_offset=None, in_=flowf[:, None],
                in_offset=IndirectOffsetOnAxis(ap=odh_s[:, c:c + 1], axis=0))
            g.indirect_dma_start(
                out=dwv[:, c:c + 1], out_offset=None, in_=flowf[:, None],
                in_offset=IndirectOffsetOnAxis(ap=odw_s[:, c:c + 1], axis=0))

        def fields(dv, hb, hb1, pfx):
            """y = (dv + hb) + 1 ; R = y - hb1 ; d = floor(R)+1-R."""
            yt = fx.tile([128, NCH], FP, tag=f"{pfx}y")
            v.tensor_tensor(out=yt[:], in0=dv[:], in1=hb[:], op=AL.add)
            v.tensor_scalar(out=yt[:], in0=yt[:], scalar1=1.0,
                            scalar2=None, op0=AL.add)
            Rt = fx.tile([128, NCH], FP, tag=f"{pfx}R")
            v.tensor_tensor(out=Rt[:], in0=yt[:], in1=hb1[:], op=AL.subtract)
            St = fx.tile([128, NCH], FP, tag=f"{pfx}S")
            gt = fx.tile([128, NCH], FP, tag=f"{pfx}g")
            v.tensor_scalar(out=St[:], in0=Rt[:], scalar1=-6.0,
                            scalar2=None, op0=AL.is_ge)
            for s in range(-5, 7):
                v.tensor_scalar(out=gt[:], in0=Rt[:], scalar1=float(s),
                                scalar2=None, op0=AL.is_ge)
                v.tensor_tensor(out=St[:], in0=St[:], in1=gt[:], op=AL.add)
            dt_ = fx.tile([128, NCH], FP, tag=f"{pfx}d")
            v.tensor_scalar(out=dt_[:], in0=St[:], scalar1=-6.0,
                            scalar2=None, op0=AL.add)
            v.tensor_tensor(out=dt_[:], in0=dt_[:], in1=Rt[:], op=AL.subtract)
            return yt, dt_

        yv, dhw = fields(dhv, oh_s, oh1_s, "fh")
        ywv, dww = fields(dwv, ow_s, ow1_s, "fw")

        # base corner in padded coords: row = clamp(y+dh+2, 0, PP-2), col same
        rowp = fx.tile([128, NCH], FP, tag="rowp")
        v.tensor_tensor(out=rowp[:], in0=yv[:], in1=dhw[:], op=AL.add)
        v.tensor_scalar(out=rowp[:], in0=rowp[:], scalar1=2.0, scalar2=0.0,
                        op0=AL.add, op1=AL.max)
        v.tensor_scalar(out=rowp[:], in0=rowp[:], scalar1=float(PP - 2),
                        scalar2=float(PP), op0=AL.min, op1=AL.mult)
        colp = fx.tile([128, NCH], FP, tag="colp")
        v.tensor_tensor(out=colp[:], in0=ywv[:], in1=dww[:], op=AL.add)
        v.tensor_scalar(out=colp[:], in0=colp[:], scalar1=2.0, scalar2=0.0,
                        op0=AL.add, op1=AL.max)
        v.tensor_scalar(out=colp[:], in0=colp[:], scalar1=float(PP - 2),
                        scalar2=None, op0=AL.min)
        af = fx.tile([128, NCH], FP, tag="af")
        v.tensor_tensor(out=af[:], in0=rowp[:], in1=colp[:], op=AL.add)
        v.tensor_tensor(out=af[:], in0=af[:], in1=obase_s[:], op=AL.add)

        vals = {}
        afo = fx.tile([128, NCH], FP, tag="afo")
        for (cn, doff) in (("v00", 0.0), ("v10", 1.0),
                           ("v01", float(PP)), ("v11", float(PP + 1))):
            ai = fx.tile([128, NCH], mybir.dt.int32, tag=f"ai{cn}")
            if doff == 0.0:
                v.tensor_copy(out=ai[:], in_=af[:])
            else:
                v.tensor_scalar(out=afo[:], in0=af[:], scalar1=doff,
                                scalar2=None, op0=AL.add)
                v.tensor_copy(out=ai[:], in_=afo[:])
            vt16 = fx.tile([128, NCH], F16, tag=f"{cn}h")
            for c in range(NCH):
                g.indirect_dma_start(
                    out=vt16[:, c:c + 1], out_offset=None, in_=pphf[:, None],
                    in_offset=IndirectOffsetOnAxis(ap=ai[:, c:c + 1], axis=0))
            vt = fx.tile([128, NCH], FP, tag=cn)
            v.tensor_copy(out=vt[:], in_=vt16[:])
            vals[cn] = vt

        omw_f = fx.tile([128, NCH], FP, tag="omwf")
        v.tensor_scalar(out=omw_f[:], in0=dww[:], scalar1=-1.0,
                        scalar2=1.0, op0=AL.mult, op1=AL.add)
        omh_f = fx.tile([128, NCH], FP, tag="omhf")
        v.tensor_scalar(out=omh_f[:], in0=dhw[:], scalar1=-1.0,
                        scalar2=1.0, op0=AL.mult, op1=AL.add)
        wt = fx.tile([128, NCH], FP, tag="wtf")
        accf = fx.tile([128, NCH], FP, tag="accf")
        t3 = fx.tile([128, NCH], FP, tag="t3")
        v.tensor_tensor(out=wt[:], in0=dhw[:], in1=dww[:], op=AL.mult)
        v.tensor_tensor(out=accf[:], in0=vals["v00"][:], in1=wt[:], op=AL.mult)
        v.tensor_tensor(out=wt[:], in0=dhw[:], in1=omw_f[:], op=AL.mult)
        v.tensor_tensor(out=t3[:], in0=vals["v10"][:], in1=wt[:], op=AL.mult)
        v.tensor_tensor(out=accf[:], in0=accf[:], in1=t3[:], op=AL.add)
        v.tensor_tensor(out=wt[:], in0=omh_f[:], in1=dww[:], op=AL.mult)
        v.tensor_tensor(out=t3[:], in0=vals["v01"][:], in1=wt[:], op=AL.mult)
        v.tensor_tensor(out=accf[:], in0=accf[:], in1=t3[:], op=AL.add)
        v.tensor_tensor(out=wt[:], in0=omw_f[:], in1=omh_f[:], op=AL.mult)
        v.tensor_tensor(out=t3[:], in0=vals["v11"][:], in1=wt[:], op=AL.mult)
        v.tensor_tensor(out=accf[:], in0=accf[:], in1=t3[:], op=AL.add)

        acc16 = fx.tile([128, NCH], F16, tag="acc16")
        v.tensor_copy(out=acc16[:], in_=accf[:])
        for c in range(NCH):
            g.indirect_dma_start(
                out=outf[:, None],
                out_offset=IndirectOffsetOnAxis(ap=opos_s[:, c:c + 1], axis=0),
                in_=acc16[:, c:c + 1], in_offset=None)


def _build_program(nout):
    nc = bacc.Bacc("TRN2", target_bir_lowering=False, debug=False,
                   enable_asserts=False, num_devices=NCORES)

    img_d = nc.dram_tensor("img", [BPC, H, W], FP, kind="ExternalInput")
    flow_d = nc.dram_tensor("flow", [BPC * 2 * HW], FP, kind="ExternalInput")
    opos_d = nc.dram_tensor("opos", [nout], mybir.dt.int32, kind="ExternalInput")
    odh_d = nc.dram_tensor("odh", [nout], mybir.dt.int32, kind="ExternalInput")
    odw_d = nc.dram_tensor("odw", [nout], mybir.dt.int32, kind="ExternalInput")
    oh_d = nc.dram_tensor("oh", [nout], FP, kind="ExternalInput")
    oh1_d = nc.dram_tensor("oh1", [nout], FP, kind="ExternalInput")
    ow_d = nc.dram_tensor("ow", [nout], FP, kind="ExternalInput")
    ow1_d = nc.dram_tensor("ow1", [nout], FP, kind="ExternalInput")
    obase_d = nc.dram_tensor("obase", [nout], FP, kind="ExternalInput")
    padh_d = nc.dram_tensor("padh", [BPC * PP * PP], F16, kind="Internal")
    dxp_d = nc.dram_tensor("dxp", [BPC * PP * PP], F16, kind="Internal")
    out_d = nc.dram_tensor("out", [BPC * HW + OUT_TAIL], F16,
                           kind="ExternalOutput")

    flowf = flow_d.ap()
    pphf = padh_d.ap()
    dxf = dxp_d.ap()
    outf = out_d.ap()
    nc._k = {
        "img": img_d.ap(),
        "flowf": flowf,
        "flow4": flowf.rearrange("(b c h w) -> b c h w", b=BPC, c=2, h=H, w=W),
        "pphf": pphf,
        "pph": pphf.rearrange("(b h w) -> b h w", b=BPC, h=PP, w=PP),
        "dxp": dxf.rearrange("(b h w) -> b h w", b=BPC, h=PP, w=PP),
        "outf": outf,
        "out3": outf[0:BPC * HW].rearrange("(b h w) -> b h w", b=BPC, h=H, w=W),
        "opos_d": opos_d, "odh_d": odh_d, "odw_d": odw_d, "oh_d": oh_d,
        "oh1_d": oh1_d, "ow_d": ow_d, "ow1_d": ow1_d, "obase_d": obase_d,
    }

    v = nc.vector
    g = nc.gpsimd

    with tile.TileContext(nc) as tc:
        with tc.tile_pool(name="pers", bufs=1) as pers:
            ident = pers.tile([128, 128], F16, tag="ident")
            make_identity(nc, ident[:])
            biases = {}
            for c in range(-3, 5):
                bt = pers.tile([128, 1], FP, tag=f"bias{c}", name=f"bias{c}")
                v.memset(bt[:], float(c))
                biases[c] = bt
            _phase_pad(nc, tc, v)
            _phase_dense(nc, tc, v, g, ident, biases)
            _phase_sparse(nc, tc, v, g, nout)

    nc.compile()
    return nc


_PROGRAM_CACHE = {}


def _get_program(nout):
    if nout not in _PROGRAM_CACHE:
        _PROGRAM_CACHE[nout] = _build_program(nout)
    return _PROGRAM_CACHE[nout]


def _host_metadata(dH, dW):
    """Outlier positions for one image, mirroring the reference fp32 math."""
    h = (np.arange(H, dtype=f32)[:, None] * np.ones((1, W), f32))
    w = (np.ones((H, 1), f32) * np.arange(W, dtype=f32)[None, :])
    y = ((dH + h).astype(f32) + f32(1.0)).astype(f32)
    yw = ((dW + w).astype(f32) + f32(1.0)).astype(f32)
    R = (y - (h + f32(1.0))).astype(f32)
    Rw = (yw - (w + f32(1.0))).astype(f32)
    inl = ((R >= f32(-4.0) + MARGIN) & (R < f32(4.0) - MARGIN)
           & (Rw >= f32(-4.0) + MARGIN) & (Rw < f32(4.0) - MARGIN))
    oy, ox = np.where(~inl)
    return oy.astype(np.int64), ox.astype(np.int64)


def _prepare(input1, input2):
    input1 = np.asarray(input1)
    input2 = np.asarray(input2)
    assert input1.shape == (B, 1, H, W) and input2.shape == (B, 2, H, W)

    metas = []
    max_n = 1
    for c in range(NCORES):
        rows = []
        for bl in range(BPC):
            bglob = c * BPC + bl
            oy, ox = _host_metadata(input2[bglob, 0], input2[bglob, 1])
            rows.append((bl, oy, ox))
        n = sum(len(oy) for _, oy, _ in rows)
        max_n = max(max_n, n)
        metas.append(rows)
    nout = max(128, ((max_n + 127) // 128) * 128)

    nc = _get_program(nout)

    in_maps = []
    for c in range(NCORES):
        imgs = input1[c * BPC:(c + 1) * BPC, 0]
        flow = input2[c * BPC:(c + 1) * BPC]
        opos = np.full(nout, BPC * HW, np.int32)
        odh = np.zeros(nout, np.int32)
        odw = np.full(nout, HW, np.int32)
        oh = np.zeros(nout, f32)
        ow = np.zeros(nout, f32)
        obase = np.zeros(nout, f32)
        k = 0
        for bl, oy, ox in metas[c]:
            n = len(oy)
            opos[k:k + n] = (bl * HW + oy * W + ox).astype(np.int32)
            odh[k:k + n] = (bl * 2 * HW + oy * W + ox).astype(np.int32)
            odw[k:k + n] = (bl * 2 * HW + HW + oy * W + ox).astype(np.int32)
            oh[k:k + n] = oy.astype(f32)
            ow[k:k + n] = ox.astype(f32)
            obase[k:k + n] = f32(bl * PP * PP)
            k += n
        in_maps.append({
            "img": np.ascontiguousarray(imgs),
            "flow": np.ascontiguousarray(flow.reshape(-1)),
            "opos": opos, "odh": odh, "odw": odw,
            "oh": oh, "oh1": (oh + f32(1.0)).astype(f32),
            "ow": ow, "ow1": (ow + f32(1.0)).astype(f32),
            "obase": obase,
        })

    return nc, in_maps


def _assemble(results):
    out = np.empty((B, 1, H, W), f32)
    for c in range(NCORES):
        o = results[c]["out"][:BPC * HW].astype(f32).reshape(BPC, H, W)
        out[c * BPC:(c + 1) * BPC, 0] = o
    return out


def kernel(input1, input2):
    nc, in_maps = _prepare(input1, input2)
    res = run_bass_kernel_spmd(nc, in_maps, core_ids=list(range(NCORES)))
    return _assemble(res.results)
